# revision 1
# baseline (speedup 1.0000x reference)
"""ClusteringLoss kernel for 8x Trainium2 NeuronCores.

Computes, for feature [8192, 512] and centroid_ids [64]:
  pd    = pairwise_distance(feature)           (torch-style, eps=1e-6)
  dc    = pd[:, centroid_ids]                  [N, K]
  facility_energy = -sum_i min_k dc[i, k]
  predictions     = argmin_k dc[i, k]
  y_fixed         = (1-mask)*predictions + constraint_vect

Only the K=64 centroid columns of pd are ever used, so the kernel computes
the [N, K] distance block directly:
  d2[i,k] = sq_i + sq_k - 2*f_i.c_k + 2*eps*(s_i - s_k) + D*eps^2
The row-constant 2*eps*s_i term (<= ~1.3e-4 absolute on d2 ~ 1e3) only
shifts all k equally, so it cannot change argmin; its effect on the energy
sum is ~1e-7 relative, far below fp32 noise. It is dropped. Everything that
varies with k (sq_k, -2*eps*s_k, D*eps^2) is kept exactly in a per-k
constant folded into the matmul.

Sharding: rows are split 1024 per core (data parallel). Each core receives
its feature shard pre-transposed ([512, 1024], contraction on partitions)
plus the replicated centroid operands, computes e[i,k] = 2*f_i.c_k - const_k
on the TensorEngine (fp32), reduces min/argmin on the VectorEngine, and
returns per-row argmin plus a per-partition sum of min distances.
"""

import numpy as np

N, D, K = 8192, 512, 64
NCORES = 8
NL = N // NCORES          # 1024 rows per core
NBLK = NL // 128          # 8 row blocks of 128
NCH = D // 128            # 4 contraction chunks of 128
EPS = 1e-6
BIG = 1024.0              # argmin tie-break offset; > K, exact in fp32

TRACE = False             # set True (e.g. from test.py) to capture an NTFF profile
LAST_EXEC_NS = None
_CACHE = {}


def _build_nc():
    import concourse.bacc as bacc
    import concourse.mybir as mybir
    import concourse.tile as tile

    f32 = mybir.dt.float32
    bf16 = mybir.dt.bfloat16

    nc = bacc.Bacc("TRN2", target_bir_lowering=False, debug=False,
                   num_devices=NCORES)
    ftd = nc.dram_tensor("ftd", [NCH, 128, NL], f32, kind="ExternalInput")
    rhs2 = nc.dram_tensor("rhs2", [128, NCH * K], f32, kind="ExternalInput")
    ncon = nc.dram_tensor("ncon", [1, NBLK * K], f32, kind="ExternalInput")
    out = nc.dram_tensor("out", [128, NBLK + 1], f32, kind="ExternalOutput")

    with tile.TileContext(nc) as tc:
        with (
            tc.tile_pool(name="const", bufs=1) as cpool,
            tc.tile_pool(name="ft", bufs=NCH) as ftpool,
            tc.tile_pool(name="fsq", bufs=NCH) as fsqpool,
            tc.tile_pool(name="ps", bufs=1, space="PSUM") as pspool,
            tc.tile_pool(name="ep", bufs=1) as eppool,
        ):
            rhs_sb = cpool.tile([128, NCH * K], f32)
            nc.sync.dma_start(rhs_sb[:, :], rhs2[:, :])
            ncon_sb = cpool.tile([1, NBLK * K], f32)
            nc.sync.dma_start(ncon_sb[:, :], ncon[:, :])
            ones1 = cpool.tile([1, 128], f32)
            nc.vector.memset(ones1[:, :], 1.0)
            onesb = cpool.tile([128, 1], bf16)
            nc.vector.memset(onesb[:, :], 1.0)
            iota_t = cpool.tile([128, NBLK * K], f32)
            nc.gpsimd.iota(iota_t[:, :].rearrange("p (b k) -> p b k", k=K),
                           pattern=[[0, NBLK], [1, K]], base=int(BIG),
                           channel_multiplier=0,
                           allow_small_or_imprecise_dtypes=True)

            # e_ps accumulates e[i,k] = 2*f_i.c_k - const_k for the 8 row
            # blocks side by side (8 x 64 cols = one 2KB PSUM bank).
            e_ps = pspool.tile([128, NBLK * K], f32)
            sq_ps = pspool.tile([128, NBLK], f32)

            # Single K=1 matmul seeds the whole bank with -const_k and opens
            # the accumulation group for every block at once.
            nc.tensor.matmul(e_ps[:, :], lhsT=ones1[:, :], rhs=ncon_sb[:, :],
                             start=True, stop=False)

            for c in range(NCH):
                ft_c = ftpool.tile([128, NL], f32)
                nc.sync.dma_start(ft_c[:, :], ftd[c, :, :])
                fsq_c = fsqpool.tile([128, NL], bf16)
                nc.scalar.activation(fsq_c[:, :], ft_c[:, :],
                                     mybir.ActivationFunctionType.Square)
                for b in range(NBLK):
                    nc.tensor.matmul(e_ps[:, b * K:(b + 1) * K],
                                     lhsT=ft_c[:, b * 128:(b + 1) * 128],
                                     rhs=rhs_sb[:, c * K:(c + 1) * K],
                                     start=False,
                                     stop=(c == NCH - 1 and b == NBLK - 1))
                for b in range(NBLK):
                    nc.tensor.matmul(sq_ps[:, b:b + 1],
                                     lhsT=fsq_c[:, b * 128:(b + 1) * 128],
                                     rhs=onesb[:, :],
                                     start=(c == 0 and b == 0),
                                     stop=(c == NCH - 1 and b == NBLK - 1))

            e3 = e_ps[:, :].rearrange("p (b k) -> p b k", k=K)
            m_sb = eppool.tile([128, NBLK], f32)
            nc.vector.tensor_reduce(m_sb[:, :], e3, axis=mybir.AxisListType.X,
                                    op=mybir.AluOpType.max)
            msk = eppool.tile([128, NBLK * K], f32)
            nc.vector.tensor_tensor(
                out=msk[:, :].rearrange("p (b k) -> p b k", k=K),
                in0=e3,
                in1=m_sb[:, :].broadcast_to((128, NBLK, K)),
                op=mybir.AluOpType.is_ge)
            sel = eppool.tile([128, NBLK * K], f32)
            nc.vector.scalar_tensor_tensor(
                out=sel[:, :], in0=msk[:, :], scalar=-BIG, in1=iota_t[:, :],
                op0=mybir.AluOpType.mult, op1=mybir.AluOpType.add)
            out_sb = eppool.tile([128, NBLK + 1], f32)
            nc.vector.tensor_reduce(out_sb[:, 0:NBLK],
                                    sel[:, :].rearrange("p (b k) -> p b k", k=K),
                                    axis=mybir.AxisListType.X,
                                    op=mybir.AluOpType.min)
            # d2min = sq_i - max_k e, clamped at 0 (torch clamps too)
            d2m = eppool.tile([128, NBLK], f32)
            nc.vector.tensor_tensor(out=d2m[:, :], in0=sq_ps[:, :],
                                    in1=m_sb[:, :],
                                    op=mybir.AluOpType.subtract)
            nc.vector.tensor_scalar_max(d2m[:, :], d2m[:, :], 0.0)
            dmin = eppool.tile([128, NBLK], f32)
            nc.scalar.activation(dmin[:, :], d2m[:, :],
                                 mybir.ActivationFunctionType.Sqrt,
                                 accum_out=out_sb[:, NBLK:NBLK + 1])
            nc.sync.dma_start(out[:, :], out_sb[:, :])

    nc.compile()
    return nc


def _get_nc():
    if "nc" not in _CACHE:
        _CACHE["nc"] = _build_nc()
    return _CACHE["nc"]


def kernel(feature, centroid_ids):
    global LAST_EXEC_NS
    from concourse.bass_utils import run_bass_kernel_spmd

    feature = np.ascontiguousarray(np.asarray(feature, dtype=np.float32))
    ids = np.asarray(centroid_ids).astype(np.int64)
    assert feature.shape == (N, D)
    assert ids.shape == (K,)

    # Deduplicate centroids (duplicate ids produce identical distance
    # columns; jnp.argmin takes the first occurrence, so duplicates can
    # never win -- map device argmin over unique centroids back to the
    # first-occurrence original index).
    ids_u, first_idx = np.unique(ids, return_index=True)
    ku = ids_u.shape[0]
    C = feature[ids_u]                                   # [ku, D]
    sq_k = (C.astype(np.float64) ** 2).sum(1)
    s_k = C.astype(np.float64).sum(1)
    const = sq_k - 2.0 * EPS * s_k + D * EPS * EPS       # [ku]
    nconst = np.full(K, -1e9, dtype=np.float32)          # padding never wins
    nconst[:ku] = (-const).astype(np.float32)

    ct2 = np.zeros((D, K), dtype=np.float32)
    ct2[:, :ku] = 2.0 * C.T
    rhs2 = np.ascontiguousarray(
        ct2.reshape(NCH, 128, K).transpose(1, 0, 2).reshape(128, NCH * K))
    ncon_in = np.tile(nconst, NBLK)[None, :]

    ft = feature.T                                       # [D, N] view
    in_maps = []
    for r in range(NCORES):
        shard = np.ascontiguousarray(ft[:, r * NL:(r + 1) * NL])
        in_maps.append({
            "ftd": shard.reshape(NCH, 128, NL),
            "rhs2": rhs2,
            "ncon": ncon_in,
        })

    nc = _get_nc()
    res = run_bass_kernel_spmd(nc, in_maps, core_ids=list(range(NCORES)),
                               trace=TRACE)
    if TRACE:
        LAST_EXEC_NS = res.exec_time_ns

    preds = np.empty(N, dtype=np.int64)
    dtot = 0.0
    for r in range(NCORES):
        o = np.asarray(res.results[r]["out"])            # [128, NBLK+1]
        preds[r * NL:(r + 1) * NL] = o[:, 0:NBLK].T.flatten().astype(np.int64)
        dtot += float(o[:, NBLK].astype(np.float64).sum())

    facility_energy = np.float32(-dtot)
    pred_orig = first_idx[preds].astype(np.float32)      # back to original k

    mask = np.zeros(N, dtype=np.float32)
    constraint = np.zeros(N, dtype=np.float32)
    mask[ids] = 1.0                                      # last-wins, like XLA scatter on CPU
    constraint[ids] = np.arange(K, dtype=np.float32)
    y_fixed = (1.0 - mask) * pred_orig + constraint
    return facility_energy, y_fixed


# revision 6
# speedup vs baseline: 1.2238x; 1.2238x over previous
"""ClusteringLoss kernel for 8x Trainium2 NeuronCores.

Computes, for feature [8192, 512] and centroid_ids [64]:
  pd    = pairwise_distance(feature)           (torch-style, eps=1e-6)
  dc    = pd[:, centroid_ids]                  [N, K]
  facility_energy = -sum_i min_k dc[i, k]
  predictions     = argmin_k dc[i, k]
  y_fixed         = (1-mask)*predictions + constraint_vect

Only the K=64 centroid columns of pd are ever used, so the kernel computes
the [N, K] distance block directly:
  d2[i,k] = sq_i + sq_k - 2*f_i.c_k + 2*eps*(s_i - s_k) + D*eps^2
The row-constant 2*eps*s_i term (<= ~1.3e-4 absolute on d2 ~ 1e3) only
shifts all k equally, so it cannot change argmin; its effect on the energy
sum is ~1e-7 relative, far below fp32 noise. It is dropped. Everything that
varies with k (sq_k, -2*eps*s_k, D*eps^2) is kept exactly in a per-k
constant folded into the matmul.

Sharding: rows are split 1024 per core (data parallel). Each core receives
its feature shard pre-transposed ([512, 1024], contraction on partitions)
plus the replicated centroid operands, computes e[i,k] = 2*f_i.c_k - const_k
on the TensorEngine (fp32), reduces min/argmin on the VectorEngine, and
returns per-row argmin plus a per-partition sum of min distances.
"""

import numpy as np

N, D, K = 8192, 512, 64
NCORES = 8
NL = N // NCORES          # 1024 rows per core
NBLK = NL // 128          # 8 row blocks of 128
NCH = D // 128            # 4 contraction chunks of 128
EPS = 1e-6
BIG = 1024.0              # argmin tie-break offset; > K, exact in fp32

TRACE = False             # set True (e.g. from test.py) to capture an NTFF profile
LAST_EXEC_NS = None
_CACHE = {}


def _build_nc():
    import concourse.bacc as bacc
    import concourse.mybir as mybir
    import concourse.tile as tile

    f32 = mybir.dt.float32
    f32r = mybir.dt.float32r
    bf16 = mybir.dt.bfloat16

    nc = bacc.Bacc("TRN2", target_bir_lowering=False, debug=False,
                   num_devices=NCORES)
    ftd = nc.dram_tensor("ftd", [NCH, 128, NL], f32r, kind="ExternalInput")
    rhs2 = nc.dram_tensor("rhs2", [128, NCH * K], f32r, kind="ExternalInput")
    ncon = nc.dram_tensor("ncon", [1, NBLK * K], f32, kind="ExternalInput")
    out = nc.dram_tensor("out", [128, NBLK + 1], f32, kind="ExternalOutput")

    with tile.TileContext(nc) as tc:
        with (
            tc.tile_pool(name="const", bufs=1) as cpool,
            tc.tile_pool(name="ft", bufs=NCH) as ftpool,
            tc.tile_pool(name="fsq", bufs=NCH) as fsqpool,
            tc.tile_pool(name="ps", bufs=1, space="PSUM") as pspool,
            tc.tile_pool(name="ep", bufs=1) as eppool,
        ):
            rhs_sb = cpool.tile([128, NCH * K], f32r)
            nc.sync.dma_start(rhs_sb[:, :], rhs2[:, :])
            ncon_sb = cpool.tile([1, NBLK * K], f32)
            nc.sync.dma_start(ncon_sb[:, :], ncon[:, :])
            ones1 = cpool.tile([1, 128], f32)
            nc.vector.memset(ones1[:, :], 1.0)
            onesb = cpool.tile([128, 1], bf16)
            nc.vector.memset(onesb[:, :], 1.0)
            iota_t = cpool.tile([128, NBLK * K], f32)
            nc.gpsimd.iota(iota_t[:, :].rearrange("p (b k) -> p b k", k=K),
                           pattern=[[0, NBLK], [1, K]], base=int(BIG),
                           channel_multiplier=0,
                           allow_small_or_imprecise_dtypes=True)

            # e_ps accumulates e[i,k] = 2*f_i.c_k - const_k for the 8 row
            # blocks side by side (8 x 64 cols = one 2KB PSUM bank).
            e_ps = pspool.tile([128, NBLK * K], f32)
            sq_ps = pspool.tile([128, NBLK], f32)

            # Single K=1 matmul seeds the whole bank with -const_k and opens
            # the accumulation group for every block at once.
            nc.tensor.matmul(e_ps[:, :], lhsT=ones1[:, :], rhs=ncon_sb[:, :],
                             start=True, stop=False)

            for c in range(NCH):
                ft_c = ftpool.tile([128, NL], f32r)
                nc.sync.dma_start(ft_c[:, :], ftd[c, :, :])
                fsq_c = fsqpool.tile([128, NL], bf16)
                nc.scalar.activation(fsq_c[:, :], ft_c[:, :].bitcast(f32),
                                     mybir.ActivationFunctionType.Square)
                for b in range(NBLK):
                    nc.tensor.matmul(e_ps[:, b * K:(b + 1) * K],
                                     lhsT=ft_c[:, b * 128:(b + 1) * 128],
                                     rhs=rhs_sb[:, c * K:(c + 1) * K],
                                     start=False,
                                     stop=(c == NCH - 1 and b == NBLK - 1))
                for b in range(NBLK):
                    nc.tensor.matmul(sq_ps[:, b:b + 1],
                                     lhsT=fsq_c[:, b * 128:(b + 1) * 128],
                                     rhs=onesb[:, :],
                                     start=(c == 0 and b == 0),
                                     stop=(c == NCH - 1 and b == NBLK - 1))

            e3 = e_ps[:, :].rearrange("p (b k) -> p b k", k=K)
            m_sb = eppool.tile([128, NBLK], f32)
            nc.vector.tensor_reduce(m_sb[:, :], e3, axis=mybir.AxisListType.X,
                                    op=mybir.AluOpType.max)
            msk = eppool.tile([128, NBLK * K], f32)
            nc.vector.tensor_tensor(
                out=msk[:, :].rearrange("p (b k) -> p b k", k=K),
                in0=e3,
                in1=m_sb[:, :].broadcast_to((128, NBLK, K)),
                op=mybir.AluOpType.is_ge)
            sel = eppool.tile([128, NBLK * K], f32)
            nc.vector.scalar_tensor_tensor(
                out=sel[:, :], in0=msk[:, :], scalar=-BIG, in1=iota_t[:, :],
                op0=mybir.AluOpType.mult, op1=mybir.AluOpType.add)
            out_sb = eppool.tile([128, NBLK + 1], f32)
            nc.vector.tensor_reduce(out_sb[:, 0:NBLK],
                                    sel[:, :].rearrange("p (b k) -> p b k", k=K),
                                    axis=mybir.AxisListType.X,
                                    op=mybir.AluOpType.min)
            # d2min = sq_i - max_k e, clamped at 0 (torch clamps too)
            d2m = eppool.tile([128, NBLK], f32)
            nc.vector.tensor_tensor(out=d2m[:, :], in0=sq_ps[:, :],
                                    in1=m_sb[:, :],
                                    op=mybir.AluOpType.subtract)
            nc.vector.tensor_scalar_max(d2m[:, :], d2m[:, :], 0.0)
            dmin = eppool.tile([128, NBLK], f32)
            nc.scalar.activation(dmin[:, :], d2m[:, :],
                                 mybir.ActivationFunctionType.Sqrt,
                                 accum_out=out_sb[:, NBLK:NBLK + 1])
            nc.sync.dma_start(out[:, :], out_sb[:, :])

    nc.compile()
    return nc


def _get_nc():
    if "nc" not in _CACHE:
        _CACHE["nc"] = _build_nc()
    return _CACHE["nc"]


def kernel(feature, centroid_ids):
    global LAST_EXEC_NS
    from concourse.bass_utils import run_bass_kernel_spmd

    feature = np.ascontiguousarray(np.asarray(feature, dtype=np.float32))
    ids = np.asarray(centroid_ids).astype(np.int64)
    assert feature.shape == (N, D)
    assert ids.shape == (K,)

    # Deduplicate centroids (duplicate ids produce identical distance
    # columns; jnp.argmin takes the first occurrence, so duplicates can
    # never win -- map device argmin over unique centroids back to the
    # first-occurrence original index).
    ids_u, first_idx = np.unique(ids, return_index=True)
    ku = ids_u.shape[0]
    C = feature[ids_u]                                   # [ku, D]
    sq_k = (C.astype(np.float64) ** 2).sum(1)
    s_k = C.astype(np.float64).sum(1)
    const = sq_k - 2.0 * EPS * s_k + D * EPS * EPS       # [ku]
    nconst = np.full(K, -1e9, dtype=np.float32)          # padding never wins
    nconst[:ku] = (-const).astype(np.float32)

    ct2 = np.zeros((D, K), dtype=np.float32)
    ct2[:, :ku] = 2.0 * C.T
    rhs2 = np.ascontiguousarray(
        ct2.reshape(NCH, 128, K).transpose(1, 0, 2).reshape(128, NCH * K))
    ncon_in = np.tile(nconst, NBLK)[None, :]

    ft = feature.T                                       # [D, N] view
    in_maps = []
    for r in range(NCORES):
        shard = np.ascontiguousarray(ft[:, r * NL:(r + 1) * NL])
        in_maps.append({
            "ftd": shard.reshape(NCH, 128, NL),
            "rhs2": rhs2,
            "ncon": ncon_in,
        })

    nc = _get_nc()
    res = run_bass_kernel_spmd(nc, in_maps, core_ids=list(range(NCORES)),
                               trace=TRACE)
    if TRACE:
        LAST_EXEC_NS = res.exec_time_ns

    preds = np.empty(N, dtype=np.int64)
    dtot = 0.0
    for r in range(NCORES):
        o = np.asarray(res.results[r]["out"])            # [128, NBLK+1]
        preds[r * NL:(r + 1) * NL] = o[:, 0:NBLK].T.flatten().astype(np.int64)
        dtot += float(o[:, NBLK].astype(np.float64).sum())

    facility_energy = np.float32(-dtot)
    pred_orig = first_idx[preds].astype(np.float32)      # back to original k

    mask = np.zeros(N, dtype=np.float32)
    constraint = np.zeros(N, dtype=np.float32)
    mask[ids] = 1.0                                      # last-wins, like XLA scatter on CPU
    constraint[ids] = np.arange(K, dtype=np.float32)
    y_fixed = (1.0 - mask) * pred_orig + constraint
    return facility_energy, y_fixed


# revision 10
# speedup vs baseline: 1.2445x; 1.0169x over previous
"""ClusteringLoss kernel for 8x Trainium2 NeuronCores.

Computes, for feature [8192, 512] and centroid_ids [64]:
  pd    = pairwise_distance(feature)           (torch-style, eps=1e-6)
  dc    = pd[:, centroid_ids]                  [N, K]
  facility_energy = -sum_i min_k dc[i, k]
  predictions     = argmin_k dc[i, k]
  y_fixed         = (1-mask)*predictions + constraint_vect

Only the K=64 centroid columns of pd are ever used, so the kernel computes
the [N, K] distance block directly:
  d2[i,k] = sq_i + sq_k - 2*f_i.c_k + 2*eps*(s_i - s_k) + D*eps^2
The row-constant 2*eps*s_i term (<= ~1.3e-4 absolute on d2 ~ 1e3) only
shifts all k equally, so it cannot change argmin; its effect on the energy
sum is ~1e-7 relative, far below fp32 noise. It is dropped. Everything that
varies with k (sq_k, -2*eps*s_k, D*eps^2) is kept exactly in a per-k
constant folded into the matmul.

Sharding: rows are split 1024 per core (data parallel). Each core receives
its feature shard pre-transposed ([512, 1024], contraction on partitions)
plus the replicated centroid operands. The gram runs [k, i]-oriented
(centroids stationary, features moving, free dim 512) in float32r so the
TensorEngine streams one row per cycle; the [64, 1024] result is transposed
back to [i, k] blocks with PE transpose-mode, then VectorE reduces
min/argmin per row. Per-row min distances are square-rooted and summed per
partition on ScalarE; the host adds the 8x128 partials and applies the
centroid mask/constraint fixup (O(K) work).
"""

import numpy as np

N, D, K = 8192, 512, 64
NCORES = 8
NL = N // NCORES          # 1024 rows per core
NBLK = NL // 128          # 8 row blocks of 128
NH = 2                    # i-halves of 512 (gram moving-dim tiles)
NCH = D // 128            # 4 contraction chunks of 128
EPS = 1e-6
BIG = 1024.0              # argmin tie-break offset; > K, exact in fp32

TRACE = False             # set True (e.g. from test.py) to capture an NTFF profile
LAST_EXEC_NS = None
_CACHE = {}


def _build_nc():
    import concourse.bacc as bacc
    import concourse.mybir as mybir
    import concourse.tile as tile

    f32 = mybir.dt.float32
    f32r = mybir.dt.float32r
    bf16 = mybir.dt.bfloat16

    nc = bacc.Bacc("TRN2", target_bir_lowering=False, debug=False,
                   num_devices=NCORES)
    # ftd[c][h] = feature_shard.T[c*128:(c+1)*128, h*512:(h+1)*512]
    ftd = nc.dram_tensor("ftd", [NCH, NH, 128, 512], f32r, kind="ExternalInput")
    rhs2 = nc.dram_tensor("rhs2", [128, NCH * K], f32r, kind="ExternalInput")
    # aux row: [0:64] = -const_k, [64:576] = 1.0 (fp32: const must stay exact)
    aux = nc.dram_tensor("aux", [1, K + 512], f32, kind="ExternalInput")
    eye = nc.dram_tensor("eye", [64, 64], f32, kind="ExternalInput")
    out = nc.dram_tensor("out", [128, NBLK + 1], f32, kind="ExternalOutput")

    with tile.TileContext(nc) as tc:
        with (
            tc.tile_pool(name="const", bufs=1) as cpool,
            tc.tile_pool(name="ft", bufs=NCH * NH) as ftpool,
            tc.tile_pool(name="fsq", bufs=NCH * NH) as fsqpool,
            tc.tile_pool(name="psg", bufs=NH, space="PSUM") as psg,
            tc.tile_pool(name="pse", bufs=1, space="PSUM") as pse,
            tc.tile_pool(name="ep", bufs=1) as eppool,
        ):
            # small constants first (scalar HWDGE ring), so the aug matmuls
            # and first gram matmul are never waiting on the big loads
            aux_sb = cpool.tile([1, K + 512], f32)
            nc.scalar.dma_start(aux_sb[:, :], aux[:, :])
            rhs_sb = cpool.tile([128, NCH * K], f32r)
            nc.scalar.dma_start(rhs_sb[:, :], rhs2[:, :])
            eye_sb = cpool.tile([64, 64], f32)
            nc.scalar.dma_start(eye_sb[:, :], eye[:, :])
            onesb = cpool.tile([128, 1], bf16)
            nc.vector.memset(onesb[:, :], 1.0)
            iota_t = cpool.tile([128, NBLK * K], f32)
            nc.gpsimd.iota(iota_t[:, :].rearrange("p (b k) -> p b k", k=K),
                           pattern=[[0, NBLK], [1, K]], base=int(BIG),
                           channel_multiplier=0,
                           allow_small_or_imprecise_dtypes=True)

            # g_ps[h][k, i'] accumulates -const_k + 2*f_i.c_k for i-half h
            g_ps = [psg.tile([64, 512], f32, tag="g", name=f"g_ps{h}")
                    for h in range(NH)]
            et_ps = pse.tile([128, NBLK * K], f32, tag="et")
            sq_ps = pse.tile([128, NBLK], f32, tag="sq")

            for h in range(NH):
                nc.tensor.matmul(g_ps[h][:, :], lhsT=aux_sb[:, 0:K],
                                 rhs=aux_sb[:, K:K + 512],
                                 start=True, stop=False)

            ft_t = [[None] * NH for _ in range(NCH)]
            fsq_t = [[None] * NH for _ in range(NCH)]
            for c in range(NCH):
                for h in range(NH):
                    t = ftpool.tile([128, 512], f32r, tag="ft")
                    # alternate the two DGE paths so issue + drain overlap
                    eng = nc.sync if h == 0 else nc.gpsimd
                    eng.dma_start(t[:, :], ftd[c, h, :, :])
                    ft_t[c][h] = t
                    nc.tensor.matmul(g_ps[h][:, :],
                                     lhsT=rhs_sb[:, c * K:(c + 1) * K],
                                     rhs=t[:, :],
                                     start=False, stop=(c == NCH - 1))
                    sq = fsqpool.tile([128, 512], bf16, tag="fsq")
                    nc.scalar.activation(sq[:, :], t[:, :].bitcast(f32),
                                         mybir.ActivationFunctionType.Square)
                    fsq_t[c][h] = sq
                    for j in range(4):
                        b = h * 4 + j
                        nc.tensor.matmul(sq_ps[:, b:b + 1],
                                         lhsT=sq[:, j * 128:(j + 1) * 128],
                                         rhs=onesb[:, :],
                                         start=(c == 0 and b == 0),
                                         stop=(c == NCH - 1 and b == NBLK - 1))

            # PSUM -> SBUF, then PE transpose back to [i, k] blocks
            g_sb = eppool.tile([64, NH * 512], f32)
            for h in range(NH):
                nc.vector.tensor_copy(g_sb[:, h * 512:(h + 1) * 512],
                                      g_ps[h][:, :])
            for b in range(NBLK):
                nc.tensor.matmul(et_ps[:, b * K:(b + 1) * K],
                                 lhsT=g_sb[:, b * 128:(b + 1) * 128],
                                 rhs=eye_sb[:, :], is_transpose=True,
                                 start=(b == 0), stop=(b == NBLK - 1))

            e3 = et_ps[:, :].rearrange("p (b k) -> p b k", k=K)
            m_sb = eppool.tile([128, NBLK], f32)
            nc.vector.tensor_reduce(m_sb[:, :], e3, axis=mybir.AxisListType.X,
                                    op=mybir.AluOpType.max)
            msk = eppool.tile([128, NBLK * K], f32)
            nc.vector.tensor_tensor(
                out=msk[:, :].rearrange("p (b k) -> p b k", k=K),
                in0=e3,
                in1=m_sb[:, :].broadcast_to((128, NBLK, K)),
                op=mybir.AluOpType.is_ge)
            sel = eppool.tile([128, NBLK * K], f32)
            nc.vector.scalar_tensor_tensor(
                out=sel[:, :], in0=msk[:, :], scalar=-BIG, in1=iota_t[:, :],
                op0=mybir.AluOpType.mult, op1=mybir.AluOpType.add)
            out_sb = eppool.tile([128, NBLK + 1], f32)
            nc.vector.tensor_reduce(out_sb[:, 0:NBLK],
                                    sel[:, :].rearrange("p (b k) -> p b k", k=K),
                                    axis=mybir.AxisListType.X,
                                    op=mybir.AluOpType.min)
            # d2min = sq_i - max_k e, clamped at 0 (torch clamps too)
            d2m = eppool.tile([128, NBLK], f32)
            nc.vector.tensor_tensor(out=d2m[:, :], in0=sq_ps[:, :],
                                    in1=m_sb[:, :],
                                    op=mybir.AluOpType.subtract)
            nc.vector.tensor_scalar_max(d2m[:, :], d2m[:, :], 0.0)
            dmin = eppool.tile([128, NBLK], f32)
            nc.scalar.activation(dmin[:, :], d2m[:, :],
                                 mybir.ActivationFunctionType.Sqrt,
                                 accum_out=out_sb[:, NBLK:NBLK + 1])
            nc.sync.dma_start(out[:, :], out_sb[:, :])

    nc.compile()
    return nc


def _get_nc():
    if "nc" not in _CACHE:
        _CACHE["nc"] = _build_nc()
    return _CACHE["nc"]


def kernel(feature, centroid_ids):
    global LAST_EXEC_NS
    from concourse.bass_utils import run_bass_kernel_spmd

    feature = np.ascontiguousarray(np.asarray(feature, dtype=np.float32))
    ids = np.asarray(centroid_ids).astype(np.int64)
    assert feature.shape == (N, D)
    assert ids.shape == (K,)

    # Deduplicate centroids (duplicate ids produce identical distance
    # columns; jnp.argmin takes the first occurrence, so duplicates can
    # never win -- map device argmin over unique centroids back to the
    # first-occurrence original index).
    ids_u, first_idx = np.unique(ids, return_index=True)
    ku = ids_u.shape[0]
    C = feature[ids_u]                                   # [ku, D]
    sq_k = (C.astype(np.float64) ** 2).sum(1)
    s_k = C.astype(np.float64).sum(1)
    const = sq_k - 2.0 * EPS * s_k + D * EPS * EPS       # [ku]
    nconst = np.full(K, -1e9, dtype=np.float32)          # padding never wins
    nconst[:ku] = (-const).astype(np.float32)

    ct2 = np.zeros((D, K), dtype=np.float32)
    ct2[:, :ku] = 2.0 * C.T
    rhs2 = np.ascontiguousarray(
        ct2.reshape(NCH, 128, K).transpose(1, 0, 2).reshape(128, NCH * K))
    aux = np.ones((1, K + 512), dtype=np.float32)
    aux[0, :K] = nconst
    eye = np.eye(64, dtype=np.float32)

    ft = feature.T                                       # [D, N] view
    in_maps = []
    for r in range(NCORES):
        shard = np.ascontiguousarray(ft[:, r * NL:(r + 1) * NL])
        in_maps.append({
            "ftd": shard.reshape(NCH, 128, NH, 512).transpose(0, 2, 1, 3).copy(),
            "rhs2": rhs2,
            "aux": aux,
            "eye": eye,
        })

    nc = _get_nc()
    res = run_bass_kernel_spmd(nc, in_maps, core_ids=list(range(NCORES)),
                               trace=TRACE)
    if TRACE:
        LAST_EXEC_NS = res.exec_time_ns

    preds = np.empty(N, dtype=np.int64)
    dtot = 0.0
    for r in range(NCORES):
        o = np.asarray(res.results[r]["out"])            # [128, NBLK+1]
        preds[r * NL:(r + 1) * NL] = o[:, 0:NBLK].T.flatten().astype(np.int64)
        dtot += float(o[:, NBLK].astype(np.float64).sum())

    facility_energy = np.float32(-dtot)
    pred_orig = first_idx[preds].astype(np.float32)      # back to original k

    mask = np.zeros(N, dtype=np.float32)
    constraint = np.zeros(N, dtype=np.float32)
    mask[ids] = 1.0                                      # last-wins, like XLA scatter on CPU
    constraint[ids] = np.arange(K, dtype=np.float32)
    y_fixed = (1.0 - mask) * pred_orig + constraint
    return facility_energy, y_fixed
